# revision 1
# baseline (speedup 1.0000x reference)
"""BatchGAT Trainium2 kernel (Bass/Tile), data-parallel over the 8 subgraphs.

Per core (1 subgraph, n=1024 nodes, 8 heads, 2 GAT layers):
  - embedding gather via indirect DMA from the replicated 100k x 64 table
  - feature-major dataflow: xT [f, n] with features on partitions
  - per head: h'T = w_h^T @ xT (PE);  t = tanh(h'T) (ACT);
    s_bcast[128, n] = (a_src broadcast) @ t  (PE outer-product trick);
    per j-chunk: d_col = t_chunk^T @ a_dst (PE);
    numerator chunks attnT[j, i]: Lrelu(s_bcast + d_col) (ACT, bias fused),
    Exp (ACT), mask-mul with adjT (DVE);
    outT = h_aug^T @ numerator (PE; ones-column yields softmax denom Z).
  - normalization deferred: Z rows gathered via tiny PE transposes into
    column form, one batched fast reciprocal, transposed back, then a
    selection-matrix matmul broadcasts 1/Z to all output rows.
  - adj is transposed once per core (u8 -> f32 convert + 64 PE transposes)
    and reused by both layers.  Layer 1 is zero-padded to fo=32 on the host
    so both layers share one code path.
  - layer outputs are restacked feature-major via constant selection-matrix
    matmuls (PE), head mean likewise; log_softmax in node-major space.
  - all constants/weights ship in one packed [128, WCOLS] tensor (single
    DMA) to keep per-instruction semaphore fan-in within ISA limits.
"""

import numpy as np

BS, N, VOCAB, EMB, FEAT = 8, 1024, 100000, 64, 64
P = 128
NCH = N // P  # 8 node chunks
H = 8
FO = 32       # per-head output features (layer 1 zero-padded to 32)
HALF = 512    # fp32 matmul free-dim limit
# head-layers whose pre-activation runs on DVE (additive mask fused)
H_HEADS = frozenset({1, 3, 5, 7, 9, 11, 13, 15})

# wpack (f32) column layout
C_IDENT = 0            # [128,128] identity
C_B0 = 128             # 1 col, partitions 0..31
C_B1 = C_B0 + 1        # 1 col, partitions 0..15
C_NEG = C_B1 + 1       # 1 col, all partitions: -3e38
WCOLS = C_NEG + 1
# wpackr (f32r matmul weights) column layout
C_W0 = 0               # 8 heads x 32 cols, partitions 0..127
C_W1 = C_W0 + 8 * 32   # 8 heads x 2 kchunks x 32 cols
C_AP0 = C_W1 + 8 * 64  # 8 heads x 2 cols (a_src, a_dst), partitions 0..31
C_AP1 = C_AP0 + 16
C_MW = C_AP1 + 16      # 16 cols, partitions 0..31 (head-mean /8)
C_SEL = C_MW + 16      # 256 cols, partitions 0..7 (1/Z row select)
C_SBLK = C_SEL + 256   # 4 x 128 cols, partitions 0..31 (x1T stacking)
RCOLS = C_SBLK + 512

_CACHE = {}


def _build(zero_b0):
    import concourse.bass as bass
    import concourse.tile as tile
    from concourse import bacc, mybir
    from contextlib import ExitStack

    dt = mybir.dt
    f32 = dt.float32
    f32r = dt.float32r
    bf16 = dt.bfloat16
    A = mybir.ActivationFunctionType
    OP = mybir.AluOpType

    nc = bacc.Bacc("TRN2", target_bir_lowering=False, debug=False)

    x_d = nc.dram_tensor("x", [P, NCH * FEAT], f32, kind="ExternalInput")
    v_d = nc.dram_tensor("verts", [P, NCH], dt.int32, kind="ExternalInput")
    adj_d = nc.dram_tensor("adj", [N, N], dt.uint8, kind="ExternalInput")
    emb_d = nc.dram_tensor("emb_w", [VOCAB, EMB], f32, kind="ExternalInput")
    wp_d = nc.dram_tensor("wpack", [P, WCOLS], f32, kind="ExternalInput")
    idb_d = nc.dram_tensor("identb", [P, P], dt.bfloat16, kind="ExternalInput")
    wpr_d = nc.dram_tensor("wpackr", [P, RCOLS], dt.float32r, kind="ExternalInput")
    out_d = nc.dram_tensor("out", [N, 16], f32, kind="ExternalOutput")

    with tile.TileContext(nc) as tc, ExitStack() as ctx:
        singles = ctx.enter_context(tc.tile_pool(name="singles", bufs=1))
        stage = ctx.enter_context(tc.tile_pool(name="stage", bufs=2))
        eepool = ctx.enter_context(tc.tile_pool(name="eepool", bufs=8))
        aupool = ctx.enter_context(tc.tile_pool(name="aupool", bufs=8))
        afpool = ctx.enter_context(tc.tile_pool(name="afpool", bufs=8))
        hpool = ctx.enter_context(tc.tile_pool(name="hpool", bufs=2))
        haug = ctx.enter_context(tc.tile_pool(name="haug", bufs=3))
        big = ctx.enter_context(tc.tile_pool(name="big", bufs=2))
        oupool = ctx.enter_context(tc.tile_pool(name="oupool", bufs=8))
        epi = ctx.enter_context(tc.tile_pool(name="epi", bufs=2))
        respool = ctx.enter_context(tc.tile_pool(name="respool", bufs=8))
        pbig = ctx.enter_context(tc.tile_pool(name="pbig", bufs=2, space="PSUM"))
        psmall = ctx.enter_context(tc.tile_pool(name="psmall", bufs=2, space="PSUM"))
        pattn_pool = ctx.enter_context(tc.tile_pool(name="pattn", bufs=1, space="PSUM"))

        # ---- packed constants (single DMA) ----
        wp = singles.tile([P, WCOLS], f32, tag="wp")
        nc.sync.dma_start(out=wp[:], in_=wp_d[:, :])
        ident = wp[:, C_IDENT:C_IDENT + P]
        identb = singles.tile([P, P], bf16, tag="identb")
        nc.sync.dma_start(out=identb[:], in_=idb_d[:, :])
        wpr = singles.tile([P, RCOLS], f32r, tag="wpr")
        nc.sync.dma_start(out=wpr[:], in_=wpr_d[:, :])

        # ---- stage A: x0T [128, 1024] = [x^T ; emb^T] ----
        # x arrives host-reshaped chunk-major [128, 8*64]; verts as [128, 8]
        xcols = singles.tile([P, NCH * FEAT], f32, tag="xcols")
        nc.sync.dma_start(out=xcols[:], in_=x_d[:, :])
        vts = singles.tile([P, NCH], dt.int32, tag="vts")
        nc.sync.dma_start(out=vts[:], in_=v_d[:, :])
        x0T = singles.tile([P, N], f32r, tag="x0T")
        pwarm = psmall.tile([P, P], f32, tag="sm")
        nc.tensor.matmul(out=pwarm[:], lhsT=ident, rhs=ident, start=True, stop=True)
        pwarm2 = psmall.tile([32, 256], f32, tag="sm", name="pwarm2")
        nc.tensor.matmul(out=pwarm2[:], lhsT=wpr[:, 0:32], rhs=wpr[:, 0:256], start=True, stop=True)
        for c in range(NCH):
            sl = slice(c * P, (c + 1) * P)
            ee = eepool.tile([P, EMB], f32, tag="ee")
            nc.gpsimd.indirect_dma_start(
                out=ee[:],
                out_offset=None,
                in_=emb_d[:, :],
                in_offset=bass.IndirectOffsetOnAxis(ap=vts[:, c:c + 1], axis=0),
            )
            xe = stage.tile([P, P], f32, tag="xe")
            nc.vector.tensor_copy(
                out=xe[:, 0:FEAT], in_=xcols[:, c * FEAT:(c + 1) * FEAT]
            )
            nc.vector.tensor_copy(out=xe[:, FEAT:P], in_=ee[:])
            px = psmall.tile([P, P], f32, tag="sm")
            nc.tensor.matmul(out=px[:], lhsT=xe[:], rhs=ident, start=True, stop=True)
            nc.vector.tensor_copy(out=x0T[:, sl], in_=px[:])

        # ---- stage B: adjT f32 [128, 8*1024]; chunk jc at cols jc*N ----
        adjT = singles.tile([P, NCH * N], bf16, tag="adjT")
        madjT = singles.tile([P, NCH * N], bf16, tag="madjT")
        af_list = []
        for ic in range(NCH):
            au = aupool.tile([P, N], dt.uint8, tag="au")
            nc.sync.dma_start(out=au[:], in_=adj_d[ic * P:(ic + 1) * P, :])
            af = afpool.tile([P, N], bf16, tag="af")
            nc.vector.tensor_copy(out=af[:], in_=au[:])
            af_list.append(af)
        # jc-outer so adjT chunk 0 (needed by the first attention chunk)
        # completes first
        for jc in range(NCH):
            for ic in range(NCH):
                pt = psmall.tile([P, P], f32, tag="sm")
                nc.tensor.matmul(
                    out=pt[:], lhsT=af_list[ic][:, jc * P:(jc + 1) * P],
                    rhs=identb[:], start=True, stop=True,
                )
                nc.scalar.activation(
                    out=adjT[:, jc * N + ic * P: jc * N + (ic + 1) * P],
                    in_=pt[:], func=A.Identity,
                )
                nc.scalar.activation(
                    out=madjT[:, jc * N + ic * P: jc * N + (ic + 1) * P],
                    in_=pt[:], func=A.Identity, scale=3.0e38, bias=wp[:, C_NEG:C_NEG + 1],
                )

        # ---- GAT layers (both padded to fo=32) ----
        fo = FO
        x1T = [
            singles.tile([P, N], f32r, tag=f"x1T{k}", name=f"x1T{k}")
            for k in range(2)
        ]

        xT_in = [x0T]
        msb = None
        for li in range(2):
            kch = 1 if li == 0 else 2
            c_w = C_W0 if li == 0 else C_W1
            c_ap = C_AP0 if li == 0 else C_AP1
            ou_list = []
            for h in range(H):
                apt = wpr[0:fo, c_ap + h * 2: c_ap + h * 2 + 2]
                # h'T = w_h^T @ xT   -> psum [fo, 1024]
                ph = pbig.tile([fo, N], f32, tag="big2")
                for hf in range(2):
                    fs = slice(hf * HALF, (hf + 1) * HALF)
                    for k in range(kch):
                        wcol = c_w + (h * kch + k) * fo
                        nc.tensor.matmul(
                            out=ph[:, fs],
                            lhsT=wpr[:, wcol:wcol + fo],
                            rhs=xT_in[k][:, fs],
                            start=(k == 0),
                            stop=(k == kch - 1),
                        )
                # tanh for attention scores only
                tT = hpool.tile([fo, N], f32r, tag="tT")
                nc.scalar.activation(out=tT[:], in_=ph[:], func=A.Tanh)
                # hT rows: 0..fo-1 h' (pre-tanh), fo = ones
                hT = hpool.tile([fo + 1, N], f32, tag="hT")
                nc.vector.tensor_copy(out=hT[0:fo, :], in_=ph[:])
                nc.gpsimd.memset(hT[fo:fo + 1, :], 1.0)
                # s broadcast to all 128 partitions: (a_src 1^T)^T @ tT
                psb = pbig.tile([P, N], f32, tag="big2")
                for hf in range(2):
                    fs = slice(hf * HALF, (hf + 1) * HALF)
                    nc.tensor.matmul(
                        out=psb[:, fs],
                        lhsT=apt[:, 0:1].to_broadcast([fo, P]),
                        rhs=tT[:, fs],
                        start=True, stop=True,
                    )

                gh = li * H + h
                h_path = gh in H_HEADS
                sbc = big.tile([P, N], f32, tag="sbc")
                nc.vector.tensor_copy(out=sbc[:], in_=psb[:])
                # attention: per j-chunk build numerator, accumulate output
                pat = pattn_pool.tile([fo + 1, N], f32, tag="pat")
                for jc in range(NCH):
                    # transpose of [h'; ones] chunk -> cols 0..fo,
                    # d column from t^T @ a_dst -> col fo+1
                    ptr = psmall.tile([P, fo + 2], f32, tag="sm")
                    nc.tensor.matmul(
                        out=ptr[:, 0:fo + 1],
                        lhsT=hT[:, jc * P:(jc + 1) * P],
                        rhs=wp[0:fo + 1, 0:fo + 1],
                        start=True, stop=True,
                    )
                    nc.tensor.matmul(
                        out=ptr[:, fo + 1:fo + 2],
                        lhsT=tT[:, jc * P:(jc + 1) * P].bitcast(f32),
                        rhs=apt[:, 1:2].bitcast(f32),
                        start=True, stop=True,
                    )
                    ha = haug.tile([P, fo + 2], bf16, tag="ha")
                    nc.vector.tensor_copy(out=ha[:], in_=ptr[:])

                    adjc = adjT[:, jc * N:(jc + 1) * N]
                    madjc = madjT[:, jc * N:(jc + 1) * N]
                    if h_path:
                        # x = s + d - BIG*(1-adj), lrelu via stt, then exp
                        xm = big.tile([P, N], f32, tag="lr", bufs=4)
                        nc.vector.scalar_tensor_tensor(
                            out=xm[:], in0=sbc[:], scalar=ha[:, fo + 1:fo + 2],
                            in1=madjc, op0=OP.add, op1=OP.add)
                        lm = big.tile([P, N], f32, tag="en", bufs=4)
                        nc.vector.scalar_tensor_tensor(
                            out=lm[:], in0=xm[:], scalar=0.2,
                            in1=xm[:], op0=OP.mult, op1=OP.max)
                        mk = big.tile([P, N], bf16, tag="mk", bufs=4)
                        nc.scalar.activation(out=mk[:], in_=lm[:], func=A.Exp)
                    else:
                        lr = big.tile([P, N], f32, tag="lr", bufs=4)
                        nc.scalar.activation(
                            out=lr[:], in_=sbc[:], func=A.Prelu,
                            bias=ha[:, fo + 1:fo + 2], scale=1.0, alpha=0.2,
                        )
                        en = big.tile([P, N], bf16, tag="en", bufs=4)
                        nc.scalar.activation(out=en[:], in_=lr[:], func=A.Exp)
                        mk = big.tile([P, N], bf16, tag="mk", bufs=4)
                        mask_eng = nc.vector if ((gh * NCH + jc) % 2 == 0) else nc.gpsimd
                        mask_eng.tensor_tensor(
                            out=mk[:], in0=en[:], in1=adjc, op=OP.mult,
                        )
                    for hf in range(2):
                        fs = slice(hf * HALF, (hf + 1) * HALF)
                        nc.tensor.matmul(
                            out=pat[:, fs],
                            lhsT=ha[:, 0:fo + 1],
                            rhs=mk[:, fs],
                            start=(jc == 0),
                            stop=(jc == NCH - 1),
                        )
                # evacuate: unscaled out rows + Z row (partition fo=32)
                ou = oupool.tile([fo + 1, N], f32, tag="ou")
                nc.vector.tensor_copy(out=ou[:], in_=pat[:])
                ou_list.append(ou)

            # gather Z rows into column form [128, 8*NCH] via PE transposes
            zcols = singles.tile([P, H * NCH], f32, tag=f"zcols{li}",
                                 name=f"zcols{li}")
            for c in range(NCH):
                pzc = psmall.tile([P, H], f32, tag="sm")
                for h in range(H):
                    nc.tensor.matmul(
                        out=pzc[:, h:h + 1],
                        lhsT=ou_list[h][fo:fo + 1, c * P:(c + 1) * P],
                        rhs=wp[fo:fo + 1, fo:fo + 1],
                        start=True, stop=True,
                    )
                nc.vector.tensor_copy(
                    out=zcols[:, c * H:(c + 1) * H], in_=pzc[:]
                )
            rcols = singles.tile([P, H * NCH], f32, tag=f"rcols{li}",
                                 name=f"rcols{li}")
            rscr = singles.tile([P, H * NCH], f32, tag=f"rscr{li}",
                                name=f"rscr{li}")
            nc.vector.reciprocal_approx_accurate(
                out=rcols[:], in_=zcols[:], scratch=rscr[:]
            )
            # transpose back to rows: rall [8, 1024]
            rall = singles.tile([H, N], f32r, tag=f"rall{li}", name=f"rall{li}")
            for c in range(NCH):
                prr = psmall.tile([H, P], f32, tag="sm")
                nc.tensor.matmul(
                    out=prr[:], lhsT=rcols[:, c * H:(c + 1) * H], rhs=ident,
                    start=True, stop=True,
                )
                nc.vector.tensor_copy(
                    out=rall[:, c * P:(c + 1) * P], in_=prr[:]
                )

            xr_list = []
            pm = None
            for h in range(H):
                # broadcast 1/Z_h to fo rows via selection matrix
                prb = pbig.tile([fo, N], f32, tag="big2")
                for hf in range(2):
                    fs = slice(hf * HALF, (hf + 1) * HALF)
                    nc.tensor.matmul(
                        out=prb[:, fs],
                        lhsT=wpr[0:H, C_SEL + h * fo: C_SEL + (h + 1) * fo],
                        rhs=rall[:, fs],
                        start=True, stop=True,
                    )
                y = epi.tile([fo, N], f32r, tag="y")
                nc.vector.tensor_tensor(
                    out=y[:], in0=ou_list[h][0:fo, :], in1=prb[:], op=OP.mult
                )
                if li == 0:
                    # x1 rows = elu(y + b0)
                    if not zero_b0:
                        yb = epi.tile([fo, N], f32, tag="yb")
                        nc.vector.tensor_scalar(
                            out=yb[:], in0=y[:], scalar1=wp[0:fo, C_B0:C_B0 + 1],
                            scalar2=None, op0=OP.add,
                        )
                        y = yb
                    m = epi.tile([fo, N], f32, tag="m", bufs=1)
                    nc.vector.tensor_scalar(
                        out=m[:], in0=y[:], scalar1=0.0, scalar2=None, op0=OP.min
                    )
                    e = epi.tile([fo, N], f32, tag="e", bufs=1)
                    nc.scalar.activation(out=e[:], in_=m[:], func=A.Exp)
                    xr = oupool.tile([fo, N], f32r, tag="ou", name="xr")
                    nc.vector.scalar_tensor_tensor(
                        out=xr[:], in0=e[:], scalar=-1.0, in1=y[:],
                        op0=OP.add, op1=OP.max,
                    )
                    xr_list.append(xr)
                else:
                    # head-mean accumulation: pm += mw^T @ y
                    if pm is None:
                        pm = pattn_pool.tile([16, N], f32, tag="pat")
                    for hf in range(2):
                        fs = slice(hf * HALF, (hf + 1) * HALF)
                        nc.tensor.matmul(
                            out=pm[:, fs],
                            lhsT=wpr[0:fo, C_MW:C_MW + 16],
                            rhs=y[:, fs],
                            start=(h == 0),
                            stop=(h == H - 1),
                        )
            if li == 0:
                # restack 8 x [32, N] into 2 x [128, N] via selection matmuls
                for k in range(2):
                    px1 = pattn_pool.tile([P, N], f32, tag="pat")
                    for hf in range(2):
                        fs = slice(hf * HALF, (hf + 1) * HALF)
                        for j in range(4):
                            nc.tensor.matmul(
                                out=px1[:, fs],
                                lhsT=wpr[0:fo, C_SBLK + j * P: C_SBLK + (j + 1) * P],
                                rhs=xr_list[k * 4 + j][:, fs],
                                start=(j == 0),
                                stop=(j == 3),
                            )
                    nc.vector.tensor_copy(out=x1T[k][:], in_=px1[:])
                xT_in = x1T
            else:
                msb = singles.tile([16, N], f32, tag="msb")
                nc.vector.tensor_scalar(
                    out=msb[:], in0=pm[:], scalar1=wp[0:16, C_B1:C_B1 + 1],
                    scalar2=None, op0=OP.add,
                )

        # ---- log_softmax over the 16 features, node-major ----
        for ic in range(NCH):
            pf = psmall.tile([P, 16], f32, tag="sm")
            nc.tensor.matmul(
                out=pf[:], lhsT=msb[:, ic * P:(ic + 1) * P],
                rhs=wp[0:16, 0:16],
                start=True, stop=True,
            )
            fm = stage.tile([P, 16], f32, tag="fm")
            nc.vector.tensor_copy(out=fm[:], in_=pf[:])
            nmx = stage.tile([P, 1], f32, tag="nmx")
            nc.vector.tensor_reduce(
                out=nmx[:], in_=fm[:], axis=mybir.AxisListType.X,
                op=OP.max, negate=True,
            )
            et = stage.tile([P, 16], f32, tag="et")
            se = stage.tile([P, 1], f32, tag="se")
            nc.scalar.activation(
                out=et[:], in_=fm[:], func=A.Exp, bias=nmx[:, :1],
                accum_out=se[:, :1],
            )
            lse = stage.tile([P, 1], f32, tag="lse")
            nc.scalar.activation(out=lse[:], in_=se[:], func=A.Ln)
            res = respool.tile([P, 16], f32, tag="res")
            nc.vector.tensor_scalar(
                out=res[:], in0=fm[:], scalar1=nmx[:, :1], scalar2=lse[:, :1],
                op0=OP.add, op1=OP.subtract,
            )
            nc.sync.dma_start(out=out_d[ic * P:(ic + 1) * P, :], in_=res[:])

    nc.compile()
    return nc


def _make_wpack(inputs):
    f32 = np.float32
    wpack = np.zeros((P, WCOLS), f32)
    wpack[:, C_IDENT:C_IDENT + P] = np.eye(P, dtype=f32)
    wpack[0:FO, C_B0] = np.asarray(inputs["b0"], f32).reshape(FO)
    wpack[0:16, C_B1] = np.asarray(inputs["b1"], f32).reshape(16)
    wpack[:, C_NEG] = -3.0e38
    wpr = np.zeros((P, RCOLS), f32)
    w0 = np.asarray(inputs["w0"], f32)
    for h in range(H):
        wpr[:, C_W0 + h * FO: C_W0 + (h + 1) * FO] = w0[h]
    w1 = np.asarray(inputs["w1"], f32)  # [8, 256, 16]
    for h in range(H):
        for k in range(2):
            blk = np.zeros((P, FO), f32)
            blk[:, :16] = w1[h, k * P:(k + 1) * P, :]
            wpr[:, C_W1 + (h * 2 + k) * FO: C_W1 + (h * 2 + k + 1) * FO] = blk
    a_src0 = np.asarray(inputs["a_src0"], f32)[..., 0]  # [8, 32]
    a_dst0 = np.asarray(inputs["a_dst0"], f32)[..., 0]
    a_src1 = np.asarray(inputs["a_src1"], f32)[..., 0]  # [8, 16]
    a_dst1 = np.asarray(inputs["a_dst1"], f32)[..., 0]
    for h in range(H):
        wpr[0:FO, C_AP0 + h * 2] = a_src0[h]
        wpr[0:FO, C_AP0 + h * 2 + 1] = a_dst0[h]
        wpr[0:16, C_AP1 + h * 2] = a_src1[h]
        wpr[0:16, C_AP1 + h * 2 + 1] = a_dst1[h]
    wpr[0:16, C_MW:C_MW + 16] = np.eye(16, dtype=f32) / 8.0
    wpr[0:H, C_SEL:C_SEL + H * FO] = np.kron(
        np.eye(H, dtype=f32), np.ones((1, FO), f32)
    )
    for j in range(4):
        wpr[0:FO, C_SBLK + j * P: C_SBLK + (j + 1) * P] = np.eye(
            FO, P, k=j * FO, dtype=f32
        )
    return wpack, wpr


def _prep_inputs(inputs):
    x = np.asarray(inputs["x"], np.float32)
    verts = np.asarray(inputs["vertices"]).astype(np.int32)
    adj = np.asarray(inputs["adj"]).astype(np.uint8)
    emb_w = np.ascontiguousarray(np.asarray(inputs["emb_w"], np.float32))
    wpack, wpr = _make_wpack(inputs)
    wpack = np.ascontiguousarray(wpack)
    wpr = np.ascontiguousarray(wpr)
    import ml_dtypes
    identb = np.ascontiguousarray(np.eye(P, dtype=ml_dtypes.bfloat16))
    in_maps = []
    for c in range(BS):
        in_maps.append({
            "x": np.ascontiguousarray(
                x[c].reshape(NCH, P, FEAT).transpose(1, 0, 2).reshape(P, NCH * FEAT)
            ),
            "verts": np.ascontiguousarray(
                verts[c].reshape(NCH, P).T
            ),
            "adj": np.ascontiguousarray(adj[c]),
            "emb_w": emb_w,
            "wpack": wpack,
            "identb": identb,
            "wpackr": wpr,
        })
    zero_b0 = bool(np.all(np.asarray(inputs["b0"]) == 0))
    return in_maps, zero_b0


def _run(inputs, trace=False):
    from concourse.bass_utils import run_bass_kernel_spmd

    in_maps, zero_b0 = _prep_inputs(inputs)
    key = ("prog", zero_b0)
    if key not in _CACHE:
        _CACHE[key] = _build(zero_b0)
    nc = _CACHE[key]
    res = run_bass_kernel_spmd(
        nc, in_maps, list(range(BS)), trace=trace
    )
    out = np.stack([res.results[c]["out"] for c in range(BS)], axis=0)
    return out.astype(np.float32), res


def kernel(**inputs):
    out, _ = _run(inputs, trace=False)
    return out



# revision 30
# speedup vs baseline: 1.7388x; 1.7388x over previous
"""BatchGAT Trainium2 kernel (Bass/Tile), data-parallel over the 8 subgraphs.

Per core (1 subgraph, n=1024 nodes, 8 heads, 2 GAT layers), the attention
matrix exp(leakyrelu(s_n + d_m)) is never exponentiated elementwise.
Using softmax's invariance to per-column (per-destination-node) scales:

  exp(lrelu(s+d)) = E2S[n] * max(exp(d_m)*exp(0.8 s_n), exp(0.2 d_m))

The E2S[n] column factor cancels between numerator and denominator, and
exp(d_m) is a per-partition (source node) scale folded into the matmul
lhsT (h_aug * exp(d)) during its PSUM evacuation.  What remains per
128x1024 attention chunk is ONE DVE op in the 4x perf mode:

  u = (E8S max exp(-0.8 d)_col) * adjT_chunk        (all bf16, SBUF)

followed by the bf16 numerator matmul (ones column scaled by exp(d)
yields the softmax denominator Z).  adj ships from the host as bf16 and
is transposed by the DMA crossbar (dma_start_transpose) straight into
SBUF.  x ships host-transposed; the embedding half of x0T is gathered
by indirect DMA and PE-transposed.  Normalization 1/Z is built in
column form (tiny PE gathers), reciprocal'd in one batched DVE op, and
broadcast back to rows via stride-0-lhsT matmuls against the bf16
identity; layer outputs (elu / head-mean) write their final stacked
layout directly as partition-sliced DVE/GPSIMD stores, so both layers
feed the next matmuls without restacking passes.
"""

import numpy as np

BS, N, VOCAB, EMB, FEAT = 8, 1024, 100000, 64, 64
P = 128
NCH = N // P  # 8 node chunks
H = 8
HALF = 512

# wpack (f32) columns
C_IDENT = 0            # [128,128] identity f32 (PE transpose helper)
C_B0 = C_IDENT + P     # 1 col, partitions 0..31
C_B1 = C_B0 + 1        # 1 col, partitions 0..15
WCOLS = C_B1 + 1

# wpackr (f32r) columns
C_W0 = 0               # 8 heads x 32 cols: w0[h] [128,32]
C_W1 = C_W0 + 8 * 32   # 8 heads x 2 kch x 16 cols: w1 blocks [128,16]
C_AS0 = C_W1 + 8 * 32  # 8 cols a_src0 (rows 0:32)
C_AS1 = C_AS0 + 8      # 8 cols a_src1 (rows 0:16)
C_AD0 = C_AS1 + 8      # 8 heads x 2 cols [a_dst0, -0.8*a_dst0] (rows 0:32)
C_AD1 = C_AD0 + 16     # 8 heads x 2 cols [a_dst1, -0.8*a_dst1] (rows 0:16)
RCOLS = C_AD1 + 16

# wpackb (bf16) columns
CB_ID = 0              # [128,128] identity bf16
CB_OR = CB_ID + P      # ones row: partition 0, 128 cols of 1.0
CB_ONE = CB_OR + P     # 1 col: 1.0 (partition 0)
CB_EIGHT = CB_ONE + 1  # 1 col: 8.0 (partition 0)
CB_MW = CB_EIGHT + 1   # [128,16] = vstack of 8 x I16
CB_O17 = CB_MW + 16    # 17 cols of 1.0 on partition 0
CB_W1 = CB_O17 + 17    # 8 heads x 2 kch x 16 cols: w1 blocks, bf16
BCOLS = CB_W1 + 8 * 2 * 16

_CACHE = {}


def _build(zero_b0):
    import concourse.bass as bass
    import concourse.tile as tile
    from concourse import bacc, mybir
    from contextlib import ExitStack

    dt = mybir.dt
    f32 = dt.float32
    f32r = dt.float32r
    bf16 = dt.bfloat16
    A = mybir.ActivationFunctionType
    OP = mybir.AluOpType

    nc = bacc.Bacc("TRN2", target_bir_lowering=False, debug=False)

    xt_d = nc.dram_tensor("xt", [FEAT, N], f32r, kind="ExternalInput")
    v_d = nc.dram_tensor("verts", [P, NCH], dt.int32, kind="ExternalInput")
    adjb_d = nc.dram_tensor("adjb", [N, N], bf16, kind="ExternalInput")
    emb_d = nc.dram_tensor("emb_w", [VOCAB, EMB], f32, kind="ExternalInput")
    wp_d = nc.dram_tensor("wpack", [P, WCOLS], f32, kind="ExternalInput")
    wpr_d = nc.dram_tensor("wpackr", [P, RCOLS], f32r, kind="ExternalInput")
    wpb_d = nc.dram_tensor("wpackb", [P, BCOLS], bf16, kind="ExternalInput")
    out_d = nc.dram_tensor("out", [N, 16], f32, kind="ExternalOutput")

    with tile.TileContext(nc) as tc, ExitStack() as ctx:
        singles = ctx.enter_context(tc.tile_pool(name="singles", bufs=1))
        eep = ctx.enter_context(tc.tile_pool(name="eep", bufs=4))
        hpool = ctx.enter_context(tc.tile_pool(name="hpool", bufs=2))
        epool = ctx.enter_context(tc.tile_pool(name="epool", bufs=2))
        ddpool = ctx.enter_context(tc.tile_pool(name="ddpool", bufs=4))
        hapool = ctx.enter_context(tc.tile_pool(name="hapool", bufs=6))
        upool = ctx.enter_context(tc.tile_pool(name="upool", bufs=4))
        oupool = ctx.enter_context(tc.tile_pool(name="oupool", bufs=8))
        ypool = ctx.enter_context(tc.tile_pool(name="ypool", bufs=2))
        mpool = ctx.enter_context(tc.tile_pool(name="mpool", bufs=2))
        stg = ctx.enter_context(tc.tile_pool(name="stg", bufs=3))
        pbig = ctx.enter_context(tc.tile_pool(name="pbig", bufs=2, space="PSUM"))
        pattn = ctx.enter_context(tc.tile_pool(name="pattn", bufs=1, space="PSUM"))
        psmall = ctx.enter_context(tc.tile_pool(name="psmall", bufs=2, space="PSUM"))

        # ---- constants (3 packed DMAs) ----
        wp = singles.tile([P, WCOLS], f32, tag="wp")
        nc.sync.dma_start(out=wp[:], in_=wp_d[:, :])
        wpr = singles.tile([P, RCOLS], f32r, tag="wpr")
        nc.sync.dma_start(out=wpr[:], in_=wpr_d[:, :])
        wpb = singles.tile([P, BCOLS], bf16, tag="wpb")
        nc.sync.dma_start(out=wpb[:], in_=wpb_d[:, :])
        identb = wpb[:, CB_ID:CB_ID + P]

        # ---- adjT via DMA crossbar transpose: adjT[p, jc*N+n] = adj[n, jc*128+p]
        adjT = singles.tile([P, NCH * N], bf16, tag="adjT")
        for jc in range(NCH):
            nc.sync.dma_start_transpose(
                out=adjT[:, jc * N:(jc + 1) * N],
                in_=adjb_d[:, jc * P:(jc + 1) * P],
            )

        # ---- x0T: rows 0:64 = xT (host-transposed), rows 64:128 = embT ----
        x0T = singles.tile([P, N], f32r, tag="x0T")
        nc.sync.dma_start(out=x0T[0:FEAT, :], in_=xt_d[:, :])
        vts = singles.tile([P, NCH], dt.int32, tag="vts")
        nc.sync.dma_start(out=vts[:], in_=v_d[:, :])

        # PE warmup
        pwarm = psmall.tile([16, 16], f32, tag="sm")
        nc.tensor.matmul(out=pwarm[:], lhsT=wpr[:, 0:16], rhs=wpr[:, 0:16],
                         start=True, stop=True)
        pwarm2 = psmall.tile([16, 16], f32, tag="sm", name="pwarm2")
        nc.tensor.matmul(out=pwarm2[:], lhsT=wpr[:, 0:16], rhs=wpr[:, 0:16],
                         start=True, stop=True)

        for c in range(NCH):
            ee = eep.tile([P, EMB], f32, tag="ee")
            nc.gpsimd.indirect_dma_start(
                out=ee[:],
                out_offset=None,
                in_=emb_d[:, :],
                in_offset=bass.IndirectOffsetOnAxis(ap=vts[:, c:c + 1], axis=0),
            )
            pe_t = psmall.tile([EMB, P], f32, tag="sm", name=f"pet{c}")
            nc.tensor.transpose(
                out=pe_t[:], in_=ee[:], identity=wp[:, C_IDENT:C_IDENT + P]
            )
            nc.vector.tensor_copy(
                out=x0T[FEAT:P, c * P:(c + 1) * P], in_=pe_t[:]
            )

        # ---- layers ----
        x1T = [
            singles.tile([P, N], bf16, tag=f"x1T{k}", name=f"x1T{k}")
            for k in range(2)
        ]
        msb = None

        xT_in = [x0T]
        fon = 33  # attn lhsT rows: fo outputs (+pad) + Z col at partition 32
        for li in range(2):
            fo = 32 if li == 0 else 16
            kch = 1 if li == 0 else 2
            c_w = C_W0 if li == 0 else C_W1
            wblk = 32 if li == 0 else 16
            c_as = C_AS0 if li == 0 else C_AS1
            c_ad = C_AD0 if li == 0 else C_AD1
            ou_list = []
            for h in range(H):
                # feature-major h' and tanh
                # weight blocks: L0 f32r (pairs with x0T f32r), L1 bf16
                # (pairs with x1T bf16; f32r may not mix with bf16)
                if li == 0:
                    wblks = [wpr[:, c_w + h * 32:c_w + h * 32 + fo]]
                else:
                    wblks = [
                        wpb[:, CB_W1 + (h * 2 + k) * 16:
                            CB_W1 + (h * 2 + k) * 16 + fo]
                        for k in range(2)
                    ]
                ph = pbig.tile([fo, N], f32, tag="big")
                for hf in range(2):
                    fs = slice(hf * HALF, (hf + 1) * HALF)
                    for k in range(kch):
                        nc.tensor.matmul(
                            out=ph[:, fs],
                            lhsT=wblks[k],
                            rhs=xT_in[k][:, fs],
                            start=(k == 0),
                            stop=(k == kch - 1),
                        )
                tT = hpool.tile([fo, N], f32r, tag="tT")
                nc.scalar.activation(out=tT[:], in_=ph[:], func=A.Tanh)

                # s broadcast -> E8S = exp(0.8 s) on all partitions (bf16)
                psb = pbig.tile([P, N], f32, tag="big", name="psb")
                for hf in range(2):
                    fs = slice(hf * HALF, (hf + 1) * HALF)
                    nc.tensor.matmul(
                        out=psb[:, fs],
                        lhsT=wpr[0:fo, c_as + h:c_as + h + 1].to_broadcast([fo, P]),
                        rhs=tT[:, fs],
                        start=True, stop=True,
                    )
                e8s = epool.tile([P, N], bf16, tag="e8s")
                nc.scalar.activation(out=e8s[:], in_=psb[:], func=A.Exp, scale=0.8)

                # d columns: edd[:, 2jc] = exp(d), edd[:, 2jc+1] = exp(-0.8 d)
                pdd = psmall.tile([P, 2 * NCH], f32, tag="sm", name="pdd")
                for jc in range(NCH):
                    nc.tensor.matmul(
                        out=pdd[:, 2 * jc:2 * jc + 2],
                        lhsT=tT[:, jc * P:(jc + 1) * P],
                        rhs=wpr[0:fo, c_ad + 2 * h:c_ad + 2 * h + 2],
                        start=True, stop=True,
                    )
                edd = ddpool.tile([P, 2 * NCH], f32, tag="edd")
                nc.scalar.activation(out=edd[:], in_=pdd[:], func=A.Exp)

                # attention chunks; Z column padded to partition 32 both
                # layers (PE base partitions must be 0/32/64)
                pat = pattn.tile([fon, N], f32, tag="pat")
                for jc in range(NCH):
                    # node-major h' for this chunk (+ ones column(s))
                    phn = psmall.tile([P, fon], f32, tag="sm", name="phn")
                    for k in range(kch):
                        nc.tensor.matmul(
                            out=phn[:, 0:fo],
                            lhsT=xT_in[k][:, jc * P:(jc + 1) * P],
                            rhs=wblks[k],
                            start=(k == 0),
                            stop=(k == kch - 1),
                        )
                    nc.tensor.matmul(
                        out=phn[:, fo:fon],
                        lhsT=wpb[0:1, CB_OR:CB_OR + P],
                        rhs=wpb[0:1, CB_O17:CB_O17 + (fon - fo)],
                        start=True, stop=True,
                    )
                    # ha' = phn * exp(d) (per-partition scale)
                    ha = hapool.tile([P, fon], bf16, tag="ha")
                    if jc % 2 == 0:
                        nc.vector.tensor_scalar(
                            out=ha[:], in0=phn[:],
                            scalar1=edd[:, 2 * jc:2 * jc + 1], scalar2=None,
                            op0=OP.mult,
                        )
                    else:
                        nc.scalar.activation(
                            out=ha[:], in_=phn[:], func=A.Identity,
                            scale=edd[:, 2 * jc:2 * jc + 1],
                        )
                    # u = max(E8S, exp(-0.8 d)) * adjT   (one DVE op, 4x mode)
                    u = upool.tile([P, N], bf16, tag="u")
                    nc.vector.scalar_tensor_tensor(
                        out=u[:], in0=e8s[:],
                        scalar=edd[:, 2 * jc + 1:2 * jc + 2], op0=OP.max,
                        in1=adjT[:, jc * N:(jc + 1) * N], op1=OP.mult,
                    )
                    for hf in range(2):
                        fs = slice(hf * HALF, (hf + 1) * HALF)
                        nc.tensor.matmul(
                            out=pat[:, fs],
                            lhsT=ha[:],
                            rhs=u[:, fs],
                            start=(jc == 0),
                            stop=(jc == NCH - 1),
                        )
                # evacuate numerator + Z row
                ou = oupool.tile([fon, N], bf16, tag="ou")
                if h % 2 == 0:
                    nc.scalar.activation(out=ou[:], in_=pat[:], func=A.Identity)
                else:
                    nc.vector.tensor_copy(out=ou[:], in_=pat[:])
                ou_list.append(ou)

            # ---- layer epilogue: Z cols, reciprocal, normalize ----
            zc_rhs = wpb[32:33, CB_ONE:CB_ONE + 1] if li == 0 else \
                wpb[32:33, CB_EIGHT:CB_EIGHT + 1]
            zcols = singles.tile([P, H * NCH], f32, tag=f"zcols{li}",
                                 name=f"zcols{li}")
            for c in range(NCH):
                pzc = psmall.tile([P, H], f32, tag="sm", name="pzc")
                for h in range(H):
                    nc.tensor.matmul(
                        out=pzc[:, h:h + 1],
                        lhsT=ou_list[h][32:33, c * P:(c + 1) * P],
                        rhs=zc_rhs,
                        start=True, stop=True,
                    )
                nc.vector.tensor_copy(out=zcols[:, c * H:(c + 1) * H], in_=pzc[:])
            rcols = singles.tile([P, H * NCH], f32, tag=f"rcols{li}",
                                 name=f"rcols{li}")
            rscr = singles.tile([P, H * NCH], f32, tag=f"rscr{li}",
                                name=f"rscr{li}")
            nc.vector.reciprocal_approx_accurate(
                out=rcols[:], in_=zcols[:], scratch=rscr[:]
            )
            rcolsb = singles.tile([P, H * NCH], bf16, tag=f"rcolsb{li}",
                                  name=f"rcolsb{li}")
            nc.vector.tensor_copy(out=rcolsb[:], in_=rcols[:])

            pmean = None
            for h in range(H):
                # broadcast 1/Z rows: prb[o, c*128+p] = rcols[p, c*8+h]
                prb = pbig.tile([fo, N], f32, tag="big", name="prb")
                for c in range(NCH):
                    nc.tensor.matmul(
                        out=prb[:, c * P:(c + 1) * P],
                        lhsT=rcolsb[:, c * H + h:c * H + h + 1].to_broadcast(
                            [P, fo]),
                        rhs=identb[:],
                        start=True, stop=True,
                    )
                if li == 0:
                    y = ypool.tile([fo, N], bf16, tag="y")
                    nc.vector.scalar_tensor_tensor(
                        out=y[:], in0=ou_list[h][0:fo, :], scalar=1.0,
                        op0=OP.mult, in1=prb[:], op1=OP.mult,
                    )
                    if not zero_b0:
                        yb = ypool.tile([fo, N], bf16, tag="y", name="yb")
                        nc.vector.tensor_scalar(
                            out=yb[:], in0=y[:], scalar1=wp[0:fo, C_B0:C_B0 + 1],
                            scalar2=None, op0=OP.add,
                        )
                        y = yb
                    m = mpool.tile([fo, N], bf16, tag="m")
                    nc.vector.tensor_scalar(
                        out=m[:], in0=y[:], scalar1=0.0, scalar2=None, op0=OP.min
                    )
                    e = mpool.tile([fo, N], bf16, tag="e")
                    nc.scalar.activation(out=e[:], in_=m[:], func=A.Exp)
                    # x1 = elu(y) written straight into its stacked slot
                    k, j = divmod(h, 4)
                    nc.vector.scalar_tensor_tensor(
                        out=x1T[k][32 * j:32 * (j + 1), :], in0=e[:],
                        scalar=-1.0, op0=OP.add, in1=y[:], op1=OP.max,
                    )
                else:
                    y = ypool.tile([fo, N], bf16, tag="y")
                    nc.vector.scalar_tensor_tensor(
                        out=y[:], in0=ou_list[h][0:fo, :], scalar=1.0,
                        op0=OP.mult, in1=prb[:], op1=OP.mult,
                    )
                    if pmean is None:
                        pmean = pattn.tile([16, N], f32, tag="pat",
                                           name="pmean")
                    for hf in range(2):
                        fs = slice(hf * HALF, (hf + 1) * HALF)
                        nc.tensor.matmul(
                            out=pmean[:, fs],
                            lhsT=wpb[0:16, CB_MW:CB_MW + 16],
                            rhs=y[:, fs],
                            start=(h == 0),
                            stop=(h == H - 1),
                        )
            if li == 0:
                xT_in = x1T
            else:
                msb = singles.tile([16, N], bf16, tag="msb")
                nc.scalar.activation(
                    out=msb[:], in_=pmean[:], func=A.Identity,
                    bias=wp[0:16, C_B1:C_B1 + 1],
                )

        # ---- log_softmax over 16 features, node-major ----
        for ic in range(NCH):
            pf = psmall.tile([P, 16], f32, tag="sm", name="pf")
            nc.tensor.matmul(
                out=pf[:], lhsT=msb[:, ic * P:(ic + 1) * P],
                rhs=identb[0:16, 0:16],
                start=True, stop=True,
            )
            fm = stg.tile([P, 16], f32, tag="fm")
            nc.vector.tensor_copy(out=fm[:], in_=pf[:])
            nmx = stg.tile([P, 1], f32, tag="nmx")
            nc.vector.tensor_reduce(
                out=nmx[:], in_=fm[:], axis=mybir.AxisListType.X,
                op=OP.max, negate=True,
            )
            et = stg.tile([P, 16], f32, tag="et")
            se = stg.tile([P, 1], f32, tag="se")
            nc.scalar.activation(
                out=et[:], in_=fm[:], func=A.Exp, bias=nmx[:, :1],
                accum_out=se[:, :1],
            )
            lse = stg.tile([P, 1], f32, tag="lse")
            nc.scalar.activation(out=lse[:], in_=se[:], func=A.Ln)
            res = stg.tile([P, 16], f32, tag="res")
            nc.vector.tensor_scalar(
                out=res[:], in0=fm[:], scalar1=nmx[:, :1], scalar2=lse[:, :1],
                op0=OP.add, op1=OP.subtract,
            )
            nc.sync.dma_start(out=out_d[ic * P:(ic + 1) * P, :], in_=res[:])

    nc.compile()
    return nc


def _make_wpack(inputs):
    import ml_dtypes
    f32 = np.float32
    wpack = np.zeros((P, WCOLS), f32)
    wpack[:, C_IDENT:C_IDENT + P] = np.eye(P, dtype=f32)
    wpack[0:32, C_B0] = np.asarray(inputs["b0"], f32).reshape(32)
    wpack[0:16, C_B1] = np.asarray(inputs["b1"], f32).reshape(16)

    wpr = np.zeros((P, RCOLS), f32)
    w0 = np.asarray(inputs["w0"], f32)      # [8, 128, 32]
    for h in range(H):
        wpr[:, C_W0 + h * 32: C_W0 + (h + 1) * 32] = w0[h]
    w1 = np.asarray(inputs["w1"], f32)      # [8, 256, 16]
    for h in range(H):
        for k in range(2):
            wpr[:, C_W1 + (h * 2 + k) * 16: C_W1 + (h * 2 + k + 1) * 16] = \
                w1[h, k * P:(k + 1) * P, :]
    a_src0 = np.asarray(inputs["a_src0"], f32)[..., 0]  # [8, 32]
    a_dst0 = np.asarray(inputs["a_dst0"], f32)[..., 0]
    a_src1 = np.asarray(inputs["a_src1"], f32)[..., 0]  # [8, 16]
    a_dst1 = np.asarray(inputs["a_dst1"], f32)[..., 0]
    for h in range(H):
        wpr[0:32, C_AS0 + h] = a_src0[h]
        wpr[0:16, C_AS1 + h] = a_src1[h]
        wpr[0:32, C_AD0 + 2 * h] = a_dst0[h]
        wpr[0:32, C_AD0 + 2 * h + 1] = -0.8 * a_dst0[h]
        wpr[0:16, C_AD1 + 2 * h] = a_dst1[h]
        wpr[0:16, C_AD1 + 2 * h + 1] = -0.8 * a_dst1[h]

    wpbf = np.zeros((P, BCOLS), f32)
    wpbf[:, CB_ID:CB_ID + P] = np.eye(P, dtype=f32)
    wpbf[0, CB_OR:CB_OR + P] = 1.0
    wpbf[:, CB_ONE] = 1.0
    wpbf[:, CB_EIGHT] = 8.0
    wpbf[:, CB_MW:CB_MW + 16] = np.tile(np.eye(16, dtype=f32), (8, 1))
    wpbf[0, CB_O17:CB_O17 + 17] = 1.0
    for h in range(H):
        for k in range(2):
            wpbf[:, CB_W1 + (h * 2 + k) * 16: CB_W1 + (h * 2 + k + 1) * 16] = \
                w1[h, k * P:(k + 1) * P, :]
    wpb = wpbf.astype(ml_dtypes.bfloat16)
    return wpack, wpr, wpb


def _prep_inputs(inputs):
    import ml_dtypes
    x = np.asarray(inputs["x"], np.float32)
    verts = np.asarray(inputs["vertices"]).astype(np.int32)
    adj = np.asarray(inputs["adj"])
    emb_w = np.ascontiguousarray(np.asarray(inputs["emb_w"], np.float32))
    wpack, wpr, wpb = _make_wpack(inputs)
    wpack = np.ascontiguousarray(wpack)
    wpr = np.ascontiguousarray(wpr)
    wpb = np.ascontiguousarray(wpb)
    in_maps = []
    for c in range(BS):
        in_maps.append({
            "xt": np.ascontiguousarray(x[c].T),
            "verts": np.ascontiguousarray(verts[c].reshape(NCH, P).T),
            "adjb": np.ascontiguousarray(adj[c].astype(ml_dtypes.bfloat16)),
            "emb_w": emb_w,
            "wpack": wpack,
            "wpackr": wpr,
            "wpackb": wpb,
        })
    zero_b0 = bool(np.all(np.asarray(inputs["b0"]) == 0))
    return in_maps, zero_b0


def _run(inputs, trace=False):
    from concourse.bass_utils import run_bass_kernel_spmd

    in_maps, zero_b0 = _prep_inputs(inputs)
    key = ("prog", zero_b0)
    if key not in _CACHE:
        _CACHE[key] = _build(zero_b0)
    nc = _CACHE[key]
    res = run_bass_kernel_spmd(
        nc, in_maps, list(range(BS)), trace=trace
    )
    out = np.stack([res.results[c]["out"] for c in range(BS)], axis=0)
    return out.astype(np.float32), res


def kernel(**inputs):
    out, _ = _run(inputs, trace=False)
    return out


# revision 36
# speedup vs baseline: 2.0229x; 1.1634x over previous
"""BatchGAT Trainium2 kernel (Bass/Tile), data-parallel over the 8 subgraphs.

Per core (1 subgraph, n=1024 nodes, 8 heads, 2 GAT layers), the attention
matrix exp(leakyrelu(s_n + d_m)) is never exponentiated elementwise.
Using softmax's invariance to per-column (per-destination-node) scales:

  exp(lrelu(s+d)) = E2S[n] * max(exp(d_m)*exp(0.8 s_n), exp(0.2 d_m))

The E2S[n] column factor cancels between numerator and denominator, and
exp(d_m) is a per-partition (source node) scale folded into the matmul
lhsT (h_aug * exp(d)) during its PSUM evacuation.  What remains per
128x1024 attention chunk is ONE DVE op in the 4x perf mode:

  u = (E8S max exp(-0.8 d)_col) * adjT_chunk        (all bf16, SBUF)

followed by the bf16 numerator matmul (ones column scaled by exp(d)
yields the softmax denominator Z).  adj ships from the host as bf16 and
is transposed by the DMA crossbar (dma_start_transpose) straight into
SBUF.  x ships host-transposed; the embedding half of x0T is gathered
by indirect DMA and PE-transposed.  Normalization 1/Z is built in
column form (tiny PE gathers), reciprocal'd in one batched DVE op, and
broadcast back to rows via stride-0-lhsT matmuls against the bf16
identity; layer outputs (elu / head-mean) write their final stacked
layout directly as partition-sliced DVE/GPSIMD stores, so both layers
feed the next matmuls without restacking passes.
"""

import numpy as np

BS, N, VOCAB, EMB, FEAT = 8, 1024, 100000, 64, 64
P = 128
NCH = N // P  # 8 node chunks
H = 8
HALF = 512

# wpack (f32) columns
C_IDENT = 0            # [128,128] identity f32 (PE transpose helper)
C_B0 = C_IDENT + P     # 1 col, partitions 0..31
C_B1 = C_B0 + 1        # 1 col, partitions 0..15
WCOLS = C_B1 + 1

# wpackr (f32r) columns
C_W0 = 0               # 8 heads x 32 cols: w0[h] [128,32]
C_W1 = C_W0 + 8 * 32   # 8 heads x 2 kch x 16 cols: w1 blocks [128,16]
C_AS0 = C_W1 + 8 * 32  # 8 cols a_src0 (rows 0:32)
C_AS1 = C_AS0 + 8      # 8 cols a_src1 (rows 0:16)
C_AD0 = C_AS1 + 8      # 8 heads x 2 cols [a_dst0, -0.8*a_dst0] (rows 0:32)
C_AD1 = C_AD0 + 16     # 8 heads x 2 cols [a_dst1, -0.8*a_dst1] (rows 0:16)
RCOLS = C_AD1 + 16

# wpackb (bf16) columns
CB_ID = 0              # [128,128] identity bf16
CB_OR = CB_ID + P      # ones row: partition 0, 128 cols of 1.0
CB_ONE = CB_OR + P     # 1 col: 1.0 (partition 0)
CB_EIGHT = CB_ONE + 1  # 1 col: 8.0 (partition 0)
CB_MW = CB_EIGHT + 1   # [128,16] = vstack of 8 x I16
CB_O17 = CB_MW + 16    # 17 cols of 1.0 on partition 0
CB_W1 = CB_O17 + 17    # 8 heads x 2 kch x 16 cols: w1 blocks, bf16
BCOLS = CB_W1 + 8 * 2 * 16

_CACHE = {}


def _build(zero_b0):
    import concourse.bass as bass
    import concourse.tile as tile
    from concourse import bacc, mybir
    from contextlib import ExitStack

    dt = mybir.dt
    f32 = dt.float32
    f32r = dt.float32r
    bf16 = dt.bfloat16
    A = mybir.ActivationFunctionType
    OP = mybir.AluOpType

    nc = bacc.Bacc("TRN2", target_bir_lowering=False, debug=False)

    xt_d = nc.dram_tensor("xt", [FEAT, N], f32r, kind="ExternalInput")
    v_d = nc.dram_tensor("verts", [P, NCH], dt.int32, kind="ExternalInput")
    adjb_d = nc.dram_tensor("adjb", [N, N], bf16, kind="ExternalInput")
    emb_d = nc.dram_tensor("emb_w", [VOCAB, EMB], f32, kind="ExternalInput")
    wp_d = nc.dram_tensor("wpack", [P, WCOLS], f32, kind="ExternalInput")
    wpr_d = nc.dram_tensor("wpackr", [P, RCOLS], f32r, kind="ExternalInput")
    wpb_d = nc.dram_tensor("wpackb", [P, BCOLS], bf16, kind="ExternalInput")
    out_d = nc.dram_tensor("out", [N, 16], f32, kind="ExternalOutput")

    with tile.TileContext(nc) as tc, ExitStack() as ctx:
        singles = ctx.enter_context(tc.tile_pool(name="singles", bufs=1))
        eep = ctx.enter_context(tc.tile_pool(name="eep", bufs=4))
        hpool = ctx.enter_context(tc.tile_pool(name="hpool", bufs=2))
        epool = ctx.enter_context(tc.tile_pool(name="epool", bufs=2))
        ddpool = ctx.enter_context(tc.tile_pool(name="ddpool", bufs=4))
        hapool = ctx.enter_context(tc.tile_pool(name="hapool", bufs=6))
        upool = ctx.enter_context(tc.tile_pool(name="upool", bufs=6))
        oupool = ctx.enter_context(tc.tile_pool(name="oupool", bufs=8))
        ypool = ctx.enter_context(tc.tile_pool(name="ypool", bufs=2))
        mpool = ctx.enter_context(tc.tile_pool(name="mpool", bufs=2))
        stg = ctx.enter_context(tc.tile_pool(name="stg", bufs=3))
        pbig = ctx.enter_context(tc.tile_pool(name="pbig", bufs=2, space="PSUM"))
        pattn = ctx.enter_context(tc.tile_pool(name="pattn", bufs=1, space="PSUM"))
        psmall = ctx.enter_context(tc.tile_pool(name="psmall", bufs=2, space="PSUM"))

        # ---- constants (3 packed DMAs) ----
        wp = singles.tile([P, WCOLS], f32, tag="wp")
        nc.sync.dma_start(out=wp[:], in_=wp_d[:, :])
        wpr = singles.tile([P, RCOLS], f32r, tag="wpr")
        nc.sync.dma_start(out=wpr[:], in_=wpr_d[:, :])
        wpb = singles.tile([P, BCOLS], bf16, tag="wpb")
        nc.sync.dma_start(out=wpb[:], in_=wpb_d[:, :])
        identb = wpb[:, CB_ID:CB_ID + P]

        # ---- adjT via DMA crossbar transpose: adjT[p, jc*N+n] = adj[n, jc*128+p]
        adjT = singles.tile([P, NCH * N], bf16, tag="adjT")
        for jc in range(NCH):
            nc.sync.dma_start_transpose(
                out=adjT[:, jc * N:(jc + 1) * N],
                in_=adjb_d[:, jc * P:(jc + 1) * P],
            )

        # ---- x0T: rows 0:64 = xT (host-transposed), rows 64:128 = embT ----
        x0T = singles.tile([P, N], f32r, tag="x0T")
        nc.sync.dma_start(out=x0T[0:FEAT, :], in_=xt_d[:, :])
        vts = singles.tile([P, NCH], dt.int32, tag="vts")
        nc.sync.dma_start(out=vts[:], in_=v_d[:, :])

        # PE warmup
        pwarm = psmall.tile([16, 16], f32, tag="sm")
        nc.tensor.matmul(out=pwarm[:], lhsT=wpr[:, 0:16], rhs=wpr[:, 0:16],
                         start=True, stop=True)
        pwarm2 = psmall.tile([16, 16], f32, tag="sm", name="pwarm2")
        nc.tensor.matmul(out=pwarm2[:], lhsT=wpr[:, 0:16], rhs=wpr[:, 0:16],
                         start=True, stop=True)

        for c in range(NCH):
            ee = eep.tile([P, EMB], f32, tag="ee")
            nc.gpsimd.indirect_dma_start(
                out=ee[:],
                out_offset=None,
                in_=emb_d[:, :],
                in_offset=bass.IndirectOffsetOnAxis(ap=vts[:, c:c + 1], axis=0),
            )
            pe_t = psmall.tile([EMB, P], f32, tag="sm", name=f"pet{c}")
            nc.tensor.transpose(
                out=pe_t[:], in_=ee[:], identity=wp[:, C_IDENT:C_IDENT + P]
            )
            nc.vector.tensor_copy(
                out=x0T[FEAT:P, c * P:(c + 1) * P], in_=pe_t[:]
            )

        # ---- layers ----
        x1T = [
            singles.tile([P, N], bf16, tag=f"x1T{k}", name=f"x1T{k}")
            for k in range(2)
        ]
        msb = None

        xT_in = [x0T]
        fon = 33  # attn lhsT rows: fo outputs (+pad) + Z col at partition 32
        for li in range(2):
            fo = 32 if li == 0 else 16
            kch = 1 if li == 0 else 2
            c_w = C_W0 if li == 0 else C_W1
            wblk = 32 if li == 0 else 16
            c_as = C_AS0 if li == 0 else C_AS1
            c_ad = C_AD0 if li == 0 else C_AD1
            ou_list = []
            for h in range(H):
                # feature-major h' and tanh
                # weight blocks: L0 f32r (pairs with x0T f32r), L1 bf16
                # (pairs with x1T bf16; f32r may not mix with bf16)
                if li == 0:
                    wblks = [wpr[:, c_w + h * 32:c_w + h * 32 + fo]]
                else:
                    wblks = [
                        wpb[:, CB_W1 + (h * 2 + k) * 16:
                            CB_W1 + (h * 2 + k) * 16 + fo]
                        for k in range(2)
                    ]
                ph = pbig.tile([fo, N], f32, tag="big")
                for hf in range(2):
                    fs = slice(hf * HALF, (hf + 1) * HALF)
                    for k in range(kch):
                        nc.tensor.matmul(
                            out=ph[:, fs],
                            lhsT=wblks[k],
                            rhs=xT_in[k][:, fs],
                            start=(k == 0),
                            stop=(k == kch - 1),
                        )
                tT = hpool.tile([fo, N], f32r, tag="tT")
                nc.scalar.activation(out=tT[:], in_=ph[:], func=A.Tanh)

                # s broadcast -> E8S = exp(0.8 s) on all partitions (bf16)
                psb = pbig.tile([P, N], f32, tag="big", name="psb")
                for hf in range(2):
                    fs = slice(hf * HALF, (hf + 1) * HALF)
                    nc.tensor.matmul(
                        out=psb[:, fs],
                        lhsT=wpr[0:fo, c_as + h:c_as + h + 1].to_broadcast([fo, P]),
                        rhs=tT[:, fs],
                        start=True, stop=True,
                    )
                e8s = epool.tile([P, N], bf16, tag="e8s")
                nc.scalar.activation(out=e8s[:], in_=psb[:], func=A.Exp, scale=0.8)

                # d columns: edd[:, 2jc] = exp(0.2 d), edd[:, 2jc+1] = exp(0.8 d)
                pdd = psmall.tile([P, 2 * NCH], f32, tag="sm", name="pdd")
                for jc in range(NCH):
                    nc.tensor.matmul(
                        out=pdd[:, 2 * jc:2 * jc + 2],
                        lhsT=tT[:, jc * P:(jc + 1) * P],
                        rhs=wpr[0:fo, c_ad + 2 * h:c_ad + 2 * h + 2],
                        start=True, stop=True,
                    )
                edd = ddpool.tile([P, 2 * NCH], f32, tag="edd")
                nc.scalar.activation(out=edd[:], in_=pdd[:], func=A.Exp)

                # attention chunks; Z column padded to partition 32 both
                # layers (PE base partitions must be 0/32/64)
                pat = pattn.tile([fon, N], f32, tag="pat")
                for jc in range(NCH):
                    # node-major h' for this chunk (+ ones column(s))
                    phn = psmall.tile([P, fon], f32, tag="sm", name="phn")
                    for k in range(kch):
                        nc.tensor.matmul(
                            out=phn[:, 0:fo],
                            lhsT=xT_in[k][:, jc * P:(jc + 1) * P],
                            rhs=wblks[k],
                            start=(k == 0),
                            stop=(k == kch - 1),
                        )
                    nc.tensor.matmul(
                        out=phn[:, fo:fon],
                        lhsT=wpb[0:1, CB_OR:CB_OR + P],
                        rhs=wpb[0:1, CB_O17:CB_O17 + (fon - fo)],
                        start=True, stop=True,
                    )
                    # ha' = phn * exp(0.2 d) (per-partition scale)
                    ha = hapool.tile([P, fon], bf16, tag="ha")
                    nc.scalar.activation(
                        out=ha[:], in_=phn[:], func=A.Identity,
                        scale=edd[:, 2 * jc:2 * jc + 1],
                    )
                    # C = max(exp(0.8 d) * E8S, 1)  (tensor_scalar, 4x mode)
                    cc = upool.tile([P, N], bf16, tag="cc", name="cc")
                    nc.vector.tensor_scalar(
                        out=cc[:], in0=e8s[:],
                        scalar1=edd[:, 2 * jc + 1:2 * jc + 2], scalar2=1.0,
                        op0=OP.mult, op1=OP.max,
                    )
                    # u = C * adjT  (tensor_tensor, 2x mode; some on POOL)
                    u = upool.tile([P, N], bf16, tag="u")
                    ueng = nc.gpsimd if jc in (2, 5, 7) else nc.vector
                    ueng.tensor_tensor(
                        out=u[:], in0=cc[:],
                        in1=adjT[:, jc * N:(jc + 1) * N], op=OP.mult,
                    )
                    for hf in range(2):
                        fs = slice(hf * HALF, (hf + 1) * HALF)
                        nc.tensor.matmul(
                            out=pat[:, fs],
                            lhsT=ha[:],
                            rhs=u[:, fs],
                            start=(jc == 0),
                            stop=(jc == NCH - 1),
                        )
                # evacuate numerator + Z row
                ou = oupool.tile([fon, N], bf16, tag="ou")
                if h % 2 == 0:
                    nc.scalar.activation(out=ou[:], in_=pat[:], func=A.Identity)
                else:
                    nc.vector.tensor_copy(out=ou[:], in_=pat[:])
                ou_list.append(ou)

            # ---- layer epilogue: Z cols, reciprocal, normalize ----
            zc_rhs = wpb[32:33, CB_ONE:CB_ONE + 1] if li == 0 else \
                wpb[32:33, CB_EIGHT:CB_EIGHT + 1]
            zcols = singles.tile([P, H * NCH], f32, tag=f"zcols{li}",
                                 name=f"zcols{li}")
            for c in range(NCH):
                pzc = psmall.tile([P, H], f32, tag="sm", name="pzc")
                for h in range(H):
                    nc.tensor.matmul(
                        out=pzc[:, h:h + 1],
                        lhsT=ou_list[h][32:33, c * P:(c + 1) * P],
                        rhs=zc_rhs,
                        start=True, stop=True,
                    )
                nc.vector.tensor_copy(out=zcols[:, c * H:(c + 1) * H], in_=pzc[:])
            rcols = singles.tile([P, H * NCH], f32, tag=f"rcols{li}",
                                 name=f"rcols{li}")
            rscr = singles.tile([P, H * NCH], f32, tag=f"rscr{li}",
                                name=f"rscr{li}")
            nc.vector.reciprocal_approx_accurate(
                out=rcols[:], in_=zcols[:], scratch=rscr[:]
            )
            rcolsb = singles.tile([P, H * NCH], bf16, tag=f"rcolsb{li}",
                                  name=f"rcolsb{li}")
            nc.vector.tensor_copy(out=rcolsb[:], in_=rcols[:])

            pmean = None
            for h in range(H):
                # broadcast 1/Z rows: prb[o, c*128+p] = rcols[p, c*8+h]
                prb = pbig.tile([fo, N], f32, tag="big", name="prb")
                for c in range(NCH):
                    nc.tensor.matmul(
                        out=prb[:, c * P:(c + 1) * P],
                        lhsT=rcolsb[:, c * H + h:c * H + h + 1].to_broadcast(
                            [P, fo]),
                        rhs=identb[:],
                        start=True, stop=True,
                    )
                if li == 0:
                    y = ypool.tile([fo, N], bf16, tag="y")
                    nc.vector.scalar_tensor_tensor(
                        out=y[:], in0=ou_list[h][0:fo, :], scalar=1.0,
                        op0=OP.mult, in1=prb[:], op1=OP.mult,
                    )
                    if not zero_b0:
                        yb = ypool.tile([fo, N], bf16, tag="y", name="yb")
                        nc.vector.tensor_scalar(
                            out=yb[:], in0=y[:], scalar1=wp[0:fo, C_B0:C_B0 + 1],
                            scalar2=None, op0=OP.add,
                        )
                        y = yb
                    m = mpool.tile([fo, N], bf16, tag="m")
                    nc.vector.tensor_scalar(
                        out=m[:], in0=y[:], scalar1=0.0, scalar2=None, op0=OP.min
                    )
                    e = mpool.tile([fo, N], bf16, tag="e")
                    nc.scalar.activation(out=e[:], in_=m[:], func=A.Exp)
                    em1 = mpool.tile([fo, N], bf16, tag="em1")
                    nc.vector.tensor_scalar(
                        out=em1[:], in0=e[:], scalar1=-1.0, scalar2=None,
                        op0=OP.add,
                    )
                    # x1 = elu(y) = max(exp(min(y,0))-1, y), straight into
                    # its stacked slot
                    k, j = divmod(h, 4)
                    nc.vector.tensor_tensor(
                        out=x1T[k][32 * j:32 * (j + 1), :], in0=em1[:],
                        in1=y[:], op=OP.max,
                    )
                else:
                    y = ypool.tile([fo, N], bf16, tag="y")
                    nc.vector.scalar_tensor_tensor(
                        out=y[:], in0=ou_list[h][0:fo, :], scalar=1.0,
                        op0=OP.mult, in1=prb[:], op1=OP.mult,
                    )
                    if pmean is None:
                        pmean = pattn.tile([16, N], f32, tag="pat",
                                           name="pmean")
                    for hf in range(2):
                        fs = slice(hf * HALF, (hf + 1) * HALF)
                        nc.tensor.matmul(
                            out=pmean[:, fs],
                            lhsT=wpb[0:16, CB_MW:CB_MW + 16],
                            rhs=y[:, fs],
                            start=(h == 0),
                            stop=(h == H - 1),
                        )
            if li == 0:
                xT_in = x1T
            else:
                msb = singles.tile([16, N], bf16, tag="msb")
                nc.scalar.activation(
                    out=msb[:], in_=pmean[:], func=A.Identity,
                    bias=wp[0:16, C_B1:C_B1 + 1],
                )

        # ---- log_softmax over 16 features, node-major ----
        for ic in range(NCH):
            pf = psmall.tile([P, 16], f32, tag="sm", name="pf")
            nc.tensor.matmul(
                out=pf[:], lhsT=msb[:, ic * P:(ic + 1) * P],
                rhs=identb[0:16, 0:16],
                start=True, stop=True,
            )
            fm = stg.tile([P, 16], f32, tag="fm")
            nc.vector.tensor_copy(out=fm[:], in_=pf[:])
            nmx = stg.tile([P, 1], f32, tag="nmx")
            nc.vector.tensor_reduce(
                out=nmx[:], in_=fm[:], axis=mybir.AxisListType.X,
                op=OP.max, negate=True,
            )
            et = stg.tile([P, 16], f32, tag="et")
            se = stg.tile([P, 1], f32, tag="se")
            nc.scalar.activation(
                out=et[:], in_=fm[:], func=A.Exp, bias=nmx[:, :1],
                accum_out=se[:, :1],
            )
            lse = stg.tile([P, 1], f32, tag="lse")
            nc.scalar.activation(out=lse[:], in_=se[:], func=A.Ln)
            res = stg.tile([P, 16], f32, tag="res")
            nc.vector.tensor_scalar(
                out=res[:], in0=fm[:], scalar1=nmx[:, :1], scalar2=lse[:, :1],
                op0=OP.add, op1=OP.subtract,
            )
            nc.sync.dma_start(out=out_d[ic * P:(ic + 1) * P, :], in_=res[:])

    nc.compile()
    return nc


def _make_wpack(inputs):
    import ml_dtypes
    f32 = np.float32
    wpack = np.zeros((P, WCOLS), f32)
    wpack[:, C_IDENT:C_IDENT + P] = np.eye(P, dtype=f32)
    wpack[0:32, C_B0] = np.asarray(inputs["b0"], f32).reshape(32)
    wpack[0:16, C_B1] = np.asarray(inputs["b1"], f32).reshape(16)

    wpr = np.zeros((P, RCOLS), f32)
    w0 = np.asarray(inputs["w0"], f32)      # [8, 128, 32]
    for h in range(H):
        wpr[:, C_W0 + h * 32: C_W0 + (h + 1) * 32] = w0[h]
    w1 = np.asarray(inputs["w1"], f32)      # [8, 256, 16]
    for h in range(H):
        for k in range(2):
            wpr[:, C_W1 + (h * 2 + k) * 16: C_W1 + (h * 2 + k + 1) * 16] = \
                w1[h, k * P:(k + 1) * P, :]
    a_src0 = np.asarray(inputs["a_src0"], f32)[..., 0]  # [8, 32]
    a_dst0 = np.asarray(inputs["a_dst0"], f32)[..., 0]
    a_src1 = np.asarray(inputs["a_src1"], f32)[..., 0]  # [8, 16]
    a_dst1 = np.asarray(inputs["a_dst1"], f32)[..., 0]
    for h in range(H):
        wpr[0:32, C_AS0 + h] = a_src0[h]
        wpr[0:16, C_AS1 + h] = a_src1[h]
        wpr[0:32, C_AD0 + 2 * h] = 0.2 * a_dst0[h]
        wpr[0:32, C_AD0 + 2 * h + 1] = 0.8 * a_dst0[h]
        wpr[0:16, C_AD1 + 2 * h] = 0.2 * a_dst1[h]
        wpr[0:16, C_AD1 + 2 * h + 1] = 0.8 * a_dst1[h]

    wpbf = np.zeros((P, BCOLS), f32)
    wpbf[:, CB_ID:CB_ID + P] = np.eye(P, dtype=f32)
    wpbf[0, CB_OR:CB_OR + P] = 1.0
    wpbf[:, CB_ONE] = 1.0
    wpbf[:, CB_EIGHT] = 8.0
    wpbf[:, CB_MW:CB_MW + 16] = np.tile(np.eye(16, dtype=f32), (8, 1))
    wpbf[0, CB_O17:CB_O17 + 17] = 1.0
    for h in range(H):
        for k in range(2):
            wpbf[:, CB_W1 + (h * 2 + k) * 16: CB_W1 + (h * 2 + k + 1) * 16] = \
                w1[h, k * P:(k + 1) * P, :]
    wpb = wpbf.astype(ml_dtypes.bfloat16)
    return wpack, wpr, wpb


def _prep_inputs(inputs):
    import ml_dtypes
    x = np.asarray(inputs["x"], np.float32)
    verts = np.asarray(inputs["vertices"]).astype(np.int32)
    adj = np.asarray(inputs["adj"])
    emb_w = np.ascontiguousarray(np.asarray(inputs["emb_w"], np.float32))
    wpack, wpr, wpb = _make_wpack(inputs)
    wpack = np.ascontiguousarray(wpack)
    wpr = np.ascontiguousarray(wpr)
    wpb = np.ascontiguousarray(wpb)
    in_maps = []
    for c in range(BS):
        in_maps.append({
            "xt": np.ascontiguousarray(x[c].T),
            "verts": np.ascontiguousarray(verts[c].reshape(NCH, P).T),
            "adjb": np.ascontiguousarray(adj[c].astype(ml_dtypes.bfloat16)),
            "emb_w": emb_w,
            "wpack": wpack,
            "wpackr": wpr,
            "wpackb": wpb,
        })
    zero_b0 = bool(np.all(np.asarray(inputs["b0"]) == 0))
    return in_maps, zero_b0


def _run(inputs, trace=False):
    from concourse.bass_utils import run_bass_kernel_spmd

    in_maps, zero_b0 = _prep_inputs(inputs)
    key = ("prog", zero_b0)
    if key not in _CACHE:
        _CACHE[key] = _build(zero_b0)
    nc = _CACHE[key]
    res = run_bass_kernel_spmd(
        nc, in_maps, list(range(BS)), trace=trace
    )
    out = np.stack([res.results[c]["out"] for c in range(BS)], axis=0)
    return out.astype(np.float32), res


def kernel(**inputs):
    out, _ = _run(inputs, trace=False)
    return out
